# revision 31
# baseline (speedup 1.0000x reference)
"""Dinov3 ViT attention (B=4, N=1024, D=1024, H=16, HD=64) on 8 TRN2
NeuronCores, written against the Bass/Tile stack.

Sharding: core c -> (batch b = c//2, head-group g = c%2, 8 heads each).
Each core computes q/k/v projections for its 512-feature slice, rotary,
attention, and a partial o_proj (its head-group's wo columns). The host
sums the two partials per batch and adds the constant bias vector
(bo + bv @ wo.T - exact, since softmax rows sum to 1).

v3 schedule.  The TileScheduler greedily drains any ready work, so the
pipeline is enforced structurally: all score/projection/V/o_proj PSUM
tiles rotate through ONE 3-slot pool (6 banks) + a separate AV
accumulator slot (2 banks).  A tile grant waits for the release three
allocations back, which ties the PE's progress to the ACT exp stream
(each scores tile is released by its exp) - emission order therefore
becomes execution order with ~3 groups of slack, with no scheduler
hints needed.

  head     chunked DMAs; pair-0 Q/K projections emitted k-major so PE
           tracks the xt chunk arrivals; K copy on ACT, Q bias on DVE.
  phase m  per kv-block: scores S^T -> exp (ACT); AV(m-1) head-A lagged
           per-kb into the pav slot; Q/K(m+1) projection half-token
           chunks as filler; AV head-B as an end-of-phase block.
  phase 3  o_proj stage-1 (pairs 0,1 -> fp16 obuf via DVE) as filler.
  tail     AV(3) both heads (pav + big slot), finalize = reciprocal +
           cross-base mult (2 DVE ops/head), o_proj stage-2 per token
           block: identity-matmul folds obuf back into PSUM (PE), adds
           pairs 2,3, ACT copies to fp16, DMA out.

Host passes pre-transposed/sliced fp16 inputs; output is fp16, host
upcasts and reduces the two per-batch partials.
"""

import sys

if "/opt/trn_rl_repo" not in sys.path:
    sys.path.insert(0, "/opt/trn_rl_repo")

import numpy as np

import concourse.bass as bass
import concourse.bacc as bacc
import concourse.mybir as mybir
from concourse import tile
from concourse import bass_utils
from contextlib import ExitStack

B, N, D = 4, 1024, 1024
H, HD = 16, 64
F = 512          # per-core feature slice (8 heads)
P = 128
NKB = 8          # contraction blocks over D
NTB = 8          # token blocks of 128
NH = 8           # local heads
MODE = "f16"

_CACHE = {}


def build_nc(mode="f16", debug=False):
    assert mode in ("f16", "bf16")
    dt = mybir.dt.float16 if mode == "f16" else mybir.dt.bfloat16
    f32 = mybir.dt.float32
    AF = mybir.ActivationFunctionType
    ALU = mybir.AluOpType

    nc = bacc.Bacc("TRN2", target_bir_lowering=False, debug=False, num_devices=8)
    xt_d = nc.dram_tensor("xt", (P, NKB * N), dt, kind="ExternalInput").ap()
    wqp_d = nc.dram_tensor("wqp", (P, 4 * NKB * P), dt, kind="ExternalInput").ap()
    wkp_d = nc.dram_tensor("wkp", (P, 4 * NKB * P), dt, kind="ExternalInput").ap()
    wvt_d = nc.dram_tensor("wvt", (P, NKB * F), dt, kind="ExternalInput").ap()
    wot_d = nc.dram_tensor("wot", (P, 4 * D), dt, kind="ExternalInput").ap()
    bq_d = nc.dram_tensor("bq", (P, 4), f32, kind="ExternalInput").ap()
    cs_d = nc.dram_tensor("cs", (P, N), dt, kind="ExternalInput").ap()
    ss_d = nc.dram_tensor("ss", (P, N), dt, kind="ExternalInput").ap()
    id_d = nc.dram_tensor("ident", (P, P), dt, kind="ExternalInput").ap()
    out_d = nc.dram_tensor("out", (N, D), dt, kind="ExternalOutput").ap()

    with tile.TileContext(nc) as tc, ExitStack() as top:
        pool = top.enter_context(tc.tile_pool(name="sb", bufs=1))

        cs_sb = pool.tile([P, N], dt, name="cs")
        ss_sb = pool.tile([P, N], dt, name="ss")
        bq_sb = pool.tile([P, 4], f32, name="bq")
        ebias = pool.tile([P, 1], f32, name="ebias")
        id_sb = pool.tile([P, P], dt, name="ident")
        xt_big = pool.tile([P, NKB * N], dt, name="xtb")
        wq_big = pool.tile([P, 4 * NKB * P], dt, name="wqb")
        wk_big = pool.tile([P, 4 * NKB * P], dt, name="wkb")
        wv_big = pool.tile([P, NKB * F], dt, name="wvb")
        wot_big = pool.tile([P, 4 * D], dt, name="wotb")
        xt_sb = [xt_big[:, k * N:(k + 1) * N] for k in range(NKB)]
        wv_sb = [wv_big[:, k * F:(k + 1) * F] for k in range(NKB)]
        wot_sb = [wot_big[:, m * D:(m + 1) * D] for m in range(4)]
        qt_sb = [pool.tile([P, N], dt, name=f"qt{m}") for m in range(4)]
        kt_sb = [pool.tile([P, N], dt, name=f"kt{m}") for m in range(4)]
        v65_sb = [pool.tile([P, NH * 128], dt, name=f"v65_{t}") for t in range(NTB)]
        ot_sb = [pool.tile([P, N], dt, name=f"ot{m}") for m in range(4)]
        obuf = [pool.tile([P, N], dt, name=f"ob{qb}") for qb in range(NTB)]

        nc.any.memset(ebias[:], -3.0)
        etbl = pool.tile([1, 1], f32, name="etbl")
        nc.scalar.activation(etbl[:], ebias[0:1, 0:1], AF.Exp)  # preload table

        # ---- DMA kickoff: xt across all 3 queues (k-major head MMs
        # track the chunk sems); gpsimd frees early for rotary swaps.
        WPC = NKB * P  # columns per pair in wq/wk packs
        nc.sync.dma_start(bq_sb[:], bq_d)
        nc.sync.dma_start(id_sb[:], id_d)
        nc.sync.dma_start(wq_big[:, 0:WPC], wqp_d[:, 0:WPC])
        nc.sync.dma_start(wk_big[:, 0:WPC], wkp_d[:, 0:WPC])
        nc.scalar.dma_start(xt_big[:, 0:3 * N], xt_d[:, 0:3 * N])
        nc.gpsimd.dma_start(xt_big[:, 3 * N:6 * N], xt_d[:, 3 * N:6 * N])
        nc.sync.dma_start(cs_sb[:], cs_d)
        nc.sync.dma_start(ss_sb[:], ss_d)
        nc.sync.dma_start(xt_big[:, 6 * N:8 * N], xt_d[:, 6 * N:8 * N])
        nc.sync.dma_start(wv_big[:], wvt_d)
        for m in range(1, 4):
            nc.scalar.dma_start(wq_big[:, m * WPC:(m + 1) * WPC],
                                wqp_d[:, m * WPC:(m + 1) * WPC])
            nc.scalar.dma_start(wk_big[:, m * WPC:(m + 1) * WPC],
                                wkp_d[:, m * WPC:(m + 1) * WPC])
        nc.scalar.dma_start(wot_big[:], wot_d)

        swp = top.enter_context(tc.tile_pool(name="swp", bufs=2))
        ptp = top.enter_context(tc.tile_pool(name="ptp", bufs=28))
        rcpp = top.enter_context(tc.tile_pool(name="rcpp", bufs=2))
        ost = top.enter_context(tc.tile_pool(name="ost", bufs=3))

        def wsl(w_big, m, k):
            return w_big[:, (m * NKB + k) * P:(m * NKB + k + 1) * P]

        def proj_half(m, which, half, big):
            """Half-token-range Q/K projection chunk for pair m (8 MMs)."""
            w_big = wq_big if which == "q" else wk_big
            dst = qt_sb[m] if which == "q" else kt_sb[m]
            hs = slice(half * F, (half + 1) * F)
            ps = big.tile([P, F], f32, tag="big", name="psp")
            for k in range(NKB):
                nc.tensor.matmul(ps[:], wsl(w_big, m, k), xt_sb[k][:, hs],
                                 start=(k == 0), stop=(k == NKB - 1))
            if which == "q":
                nc.vector.tensor_scalar_add(dst[:, hs], ps[:],
                                            bq_sb[:, m:m + 1])
            else:
                nc.vector.tensor_copy(dst[:, hs], ps[:])

        def rotary(m, which, crit=False):
            """In-place rotary on qt/kt[m]: src = src*cs + swap(src)*ss.

            crit: head critical path - swaps on 2 queues, each element-wise
            op column-split across DVE and gpsimd to halve chain latency.
            """
            src_sb = qt_sb if which == "q" else kt_sb
            sw = swp.tile([P, N], dt, tag="sw", name="sw")
            for blk in range(4):
                o = blk * 32
                eng = nc.sync if (crit and blk % 2) else nc.gpsimd
                eng.dma_start(sw[o:o + 32, :],
                              src_sb[m][o ^ 32:(o ^ 32) + 32, :])
            if crit:
                hv, hg = slice(0, F), slice(F, N)
                for dst, in0, in1 in (
                        (sw, sw, ss_sb),
                        (src_sb[m], src_sb[m], cs_sb)):
                    nc.vector.tensor_tensor(dst[:, hv], in0[:, hv],
                                            in1[:, hv], op=ALU.mult)
                    nc.gpsimd.tensor_tensor(dst[:, hg], in0[:, hg],
                                            in1[:, hg], op=ALU.mult)
                nc.vector.tensor_tensor(src_sb[m][:, hv], src_sb[m][:, hv],
                                        sw[:, hv], op=ALU.add)
                nc.gpsimd.tensor_tensor(src_sb[m][:, hg], src_sb[m][:, hg],
                                        sw[:, hg], op=ALU.add)
            else:
                nc.gpsimd.tensor_tensor(sw[:], sw[:], ss_sb[:], op=ALU.mult)
                nc.vector.tensor_tensor(src_sb[m][:], src_sb[m][:], cs_sb[:],
                                        op=ALU.mult)
                nc.vector.tensor_tensor(src_sb[m][:], src_sb[m][:], sw[:],
                                        op=ALU.add)

        def scores_kb(m, kb, big, pts):
            """S^T for pair m, kv-block kb -> exp -> pts tiles (fp16)."""
            ps2 = [big.tile([P, N], f32, tag="big", name="pss")
                   for _ in range(2)]
            for qh in range(2):
                qs = slice(qh * F, (qh + 1) * F)
                for par in range(2):
                    off = par * 64
                    nc.tensor.matmul(
                        ps2[par][:, qs],
                        kt_sb[m][off:off + 64, kb * P:(kb + 1) * P],
                        qt_sb[m][off:off + 64, qs],
                        start=True, stop=True, tile_position=(off, 0))
            for par in range(2):
                ptile = ptp.tile([P, N], dt, tag="pt", name="ptile", bufs=28)
                nc.scalar.activation(ptile[:], ps2[par][:], AF.Exp,
                                     scale=0.125, bias=ebias[:])
                pts[par][kb] = ptile

        def av_kb(m, par, kb, av, pts):
            """AV accumulation MMs for head 2m+par, kv-block kb."""
            h = 2 * m + par
            for qh in range(2):
                qs = slice(qh * F, (qh + 1) * F)
                nc.tensor.matmul(av[:, qs],
                                 v65_sb[kb][:, h * 128:(h + 1) * 128],
                                 pts[par][kb][:, qs],
                                 start=(kb == 0), stop=(kb == NTB - 1))

        def av_finalize(m, par, av):
            """ot rows for head 2m+par = av[64:128] * recip(av[0:64])."""
            off = par * 64
            rcp = rcpp.tile([64, N], f32, tag="rcp", name="rcp")
            nc.vector.reciprocal_approx_fast(rcp[:], av[0:64, :])
            nc.vector.tensor_tensor(ot_sb[m][off:off + 64, :],
                                    av[64:128, :], rcp[:], op=ALU.mult)

        def v_proj(t, big):
            """V projection for token block t -> v65 (128-wide head slots)."""
            ps = big.tile([P, F], f32, tag="big", name="psv")
            for k in range(NKB):
                nc.tensor.matmul(ps[:], xt_sb[k][:, t * P:(t + 1) * P],
                                 wv_sb[k][:], start=(k == 0),
                                 stop=(k == NKB - 1))
            nc.any.memset(v65_sb[t][:], 1.0)
            dst = v65_sb[t][:].rearrange("p (h e) -> p h e", e=128)[:, :, 64:128]
            nc.vector.tensor_copy(dst, ps[:].rearrange("p (h e) -> p h e", e=64))

        def oproj_stage1(qb, big):
            """o_proj partial (pairs 0,1) for token block qb -> fp16 obuf."""
            ps = big.tile([P, D], f32, tag="big", name="ps1")
            for i, mm_ in enumerate((0, 1)):
                for half in range(2):
                    hs = slice(half * F, (half + 1) * F)
                    nc.tensor.matmul(ps[:, hs],
                                     ot_sb[mm_][:, qb * P:(qb + 1) * P],
                                     wot_sb[mm_][:, hs],
                                     start=(i == 0), stop=(i == 1))
            if qb >= 5:
                nc.scalar.copy(obuf[qb][:], ps[:])
            else:
                nc.vector.tensor_copy(obuf[qb][:], ps[:])

        def oproj_s2a(qb, big):
            """Stage-2 preload: obuf (identity-MM) + pair 2 into PSUM."""
            ps = big.tile([P, D], f32, tag="big", name="ps2")
            for half in range(2):
                hs = slice(half * F, (half + 1) * F)
                nc.tensor.matmul(ps[:, hs], id_sb[:], obuf[qb][:, hs],
                                 start=True, stop=False)
            for half in range(2):
                hs = slice(half * F, (half + 1) * F)
                nc.tensor.matmul(ps[:, hs],
                                 ot_sb[2][:, qb * P:(qb + 1) * P],
                                 wot_sb[2][:, hs], start=False, stop=False)
            return ps

        def oproj_s2b(qb, ps):
            """Stage-2 finish: pair 3, ACT copy to fp16, DMA out."""
            for half in range(2):
                hs = slice(half * F, (half + 1) * F)
                nc.tensor.matmul(ps[:, hs],
                                 ot_sb[3][:, qb * P:(qb + 1) * P],
                                 wot_sb[3][:, hs], start=False, stop=True)
            oout = ost.tile([P, D], dt, tag="oout", name="oout")
            if qb % 2 == 0:
                nc.scalar.copy(oout[:], ps[:])
            else:
                nc.vector.tensor_copy(oout[:], ps[:])
            eng = nc.sync if qb % 2 == 0 else nc.scalar
            eng.dma_start(out_d[qb * P:(qb + 1) * P, :], oout[:])

        pts = [[[None] * NTB, [None] * NTB] for _ in range(4)]

        with ExitStack() as st_att:
            big = st_att.enter_context(tc.tile_pool(name="big", bufs=3,
                                                    space="PSUM"))
            pav = st_att.enter_context(tc.tile_pool(name="pav", bufs=1,
                                                    space="PSUM"))

            # ---------- head: pair-0 Q/K projections, k-major ----------
            psQ = big.tile([P, N], f32, tag="big", name="psQ0")
            psK = big.tile([P, N], f32, tag="big", name="psK0")
            for k in range(NKB):
                for half in range(2):
                    hs = slice(half * F, (half + 1) * F)
                    nc.tensor.matmul(psQ[:, hs], wsl(wq_big, 0, k),
                                     xt_sb[k][:, hs],
                                     start=(k == 0), stop=(k == NKB - 1))
                    nc.tensor.matmul(psK[:, hs], wsl(wk_big, 0, k),
                                     xt_sb[k][:, hs],
                                     start=(k == 0), stop=(k == NKB - 1))
            nc.vector.tensor_scalar_add(qt_sb[0][:], psQ[:], bq_sb[:, 0:1])
            nc.scalar.copy(kt_sb[0][:], psK[:])
            rotary(0, "q", crit=True)
            rotary(0, "k", crit=True)

            # ---------- phase 0: scores(0) + V proj t0-5 + QK(1) ----------
            # V t0/t1 ahead of the first scores: fills PE during the
            # pair-0 rotary chain instead of head-of-line blocking on it.
            v_proj(0, big)
            v_proj(1, big)
            for kb in range(NTB):
                scores_kb(0, kb, big, pts[0])
                if 2 <= kb < 6:
                    v_proj(kb, big)
                if kb == 0:
                    proj_half(1, "q", 0, big)
                elif kb == 1:
                    proj_half(1, "q", 1, big)
                elif kb == 2:
                    rotary(1, "q")
                    proj_half(1, "k", 0, big)
                elif kb == 3:
                    proj_half(1, "k", 1, big)
                elif kb == 4:
                    rotary(1, "k")

            # ---------- phases 1..3 ----------
            # AVA(m-1) lags 3 kv-blocks so its pav grant (freed by the
            # previous AVB finalize, which lands ~2 groups into the phase)
            # never head-of-line blocks PE.  The AVB(m-1) block is emitted
            # AFTER the next phase's first two scores groups, so the ACT
            # stream crosses phase boundaries without starving.
            def fillers(m, j):
                # j = position within the emitted body (0-based)
                if m == 1 and j in (0, 2):
                    v_proj(6 + j // 2, big)
                if m == 3:
                    if 1 <= j <= 4:
                        oproj_stage1(2 * (j - 1), big)
                        oproj_stage1(2 * (j - 1) + 1, big)
                    return
                if j == 0:
                    proj_half(m + 1, "q", 0, big)
                elif j == 1:
                    proj_half(m + 1, "q", 1, big)
                elif j == 2:
                    rotary(m + 1, "q")
                    proj_half(m + 1, "k", 0, big)
                elif j == 3:
                    proj_half(m + 1, "k", 1, big)
                elif j == 4:
                    rotary(m + 1, "k")

            avA = {0: pav.tile([P, N], f32, tag="pav", name="avA0")}
            for m in (1, 2, 3):
                start_kb = 0 if m == 1 else 2
                for kb in range(start_kb, NTB):
                    scores_kb(m, kb, big, pts[m])
                    if kb >= 3:
                        av_kb(m - 1, 0, kb - 3, avA[m - 1], pts[m - 1])
                    fillers(m, kb - start_kb)
                for k2 in (5, 6, 7):
                    av_kb(m - 1, 0, k2, avA[m - 1], pts[m - 1])
                av_finalize(m - 1, 0, avA[m - 1])
                if m < 3:
                    scores_kb(m + 1, 0, big, pts[m + 1])
                    scores_kb(m + 1, 1, big, pts[m + 1])
                    avB = pav.tile([P, N], f32, tag="pav", name=f"avB{m-1}")
                    for k2 in range(NTB):
                        av_kb(m - 1, 1, k2, avB, pts[m - 1])
                    av_finalize(m - 1, 1, avB)
                    avA[m] = pav.tile([P, N], f32, tag="pav", name=f"avA{m}")

            # ---------- tail: AV(3) + o_proj stage-2 ----------
            # AV(3) accumulators both in the big pool: their MMs are
            # pts-gated only, so they run concurrently with the AVB(2)
            # block (pav) instead of serializing behind its finalize.
            avA3 = big.tile([P, N], f32, tag="big", name="avA4")
            avB3 = big.tile([P, N], f32, tag="big", name="avB4")
            av_kb(3, 0, 0, avA3, pts[3])
            av_kb(3, 1, 0, avB3, pts[3])
            avB2 = pav.tile([P, N], f32, tag="pav", name="avB2b")
            for k2 in range(NTB):
                av_kb(2, 1, k2, avB2, pts[2])
            av_finalize(2, 1, avB2)
            for kb in range(1, NTB - 1):
                av_kb(3, 0, kb, avA3, pts[3])
                av_kb(3, 1, kb, avB3, pts[3])
            av_kb(3, 0, NTB - 1, avA3, pts[3])
            av_finalize(3, 0, avA3)
            av_kb(3, 1, NTB - 1, avB3, pts[3])
            av_finalize(3, 1, avB3)
            ps2 = {0: oproj_s2a(0, big)}
            for qb in range(NTB):
                oproj_s2b(qb, ps2.pop(qb))
                if qb + 1 < NTB:
                    ps2[qb + 1] = oproj_s2a(qb + 1, big)

    nc.compile()
    return nc


def host_prep(inputs, mode=MODE):
    """Slice/transpose full inputs into 8 per-core input maps."""
    hs = np.asarray(inputs["hidden_states"], np.float32)
    cos = np.asarray(inputs["cos"], np.float32)
    sin = np.asarray(inputs["sin"], np.float32)
    wq = np.asarray(inputs["wq"], np.float32)
    wk = np.asarray(inputs["wk"], np.float32)
    wv = np.asarray(inputs["wv"], np.float32)
    wo = np.asarray(inputs["wo"], np.float32)
    bq = np.asarray(inputs["bq"], np.float32)

    if mode == "bf16":
        import ml_dtypes
        cast = lambda a: np.ascontiguousarray(a).astype(ml_dtypes.bfloat16)
    else:
        cast = lambda a: np.ascontiguousarray(a).astype(np.float16)

    sgn = np.ones((64, 1), np.float32)
    sgn[:32] = -1.0
    ident = np.eye(P, dtype=np.float32)
    in_maps = []
    for c in range(8):
        b, g = c // 2, c % 2
        fs = slice(g * F, (g + 1) * F)
        csx = cos[b].T  # (64, N)
        ssx = sin[b].T * sgn
        pack = lambda a, kd: np.ascontiguousarray(
            a.reshape(kd, P, -1).transpose(1, 0, 2).reshape(P, -1))

        def pack_pairs(w):
            # w: (512 out-features, 1024 D) -> [P, 4*NKB*P] pair-major
            wt = w.T  # (1024 D, 512 feat)
            blocks = []
            for m in range(4):
                blocks.append(pack(wt[:, m * P:(m + 1) * P], NKB))
            return np.concatenate(blocks, axis=1)

        in_maps.append({
            "xt": cast(pack(hs[b].T, NKB)),
            "wqp": cast(pack_pairs(wq[fs, :])),
            "wkp": cast(pack_pairs(wk[fs, :])),
            "wvt": cast(pack(wv[fs, :].T, NKB)),
            "wot": cast(pack(wo[:, fs].T, 4)),
            "bq": np.ascontiguousarray(bq[fs].reshape(4, P).T, np.float32),
            "cs": cast(np.concatenate([csx, csx], axis=0)),
            "ss": cast(np.concatenate([ssx, ssx], axis=0)),
            "ident": cast(ident),
        })
    return in_maps


def host_finish(results, inputs):
    bo = np.asarray(inputs["bo"], np.float32)
    bv = np.asarray(inputs["bv"], np.float32)
    wo = np.asarray(inputs["wo"], np.float32)
    const = bo + bv @ wo.T
    out = np.empty((B, N, D), np.float32)
    for b in range(B):
        out[b] = (results[2 * b]["out"].astype(np.float32)
                  + results[2 * b + 1]["out"].astype(np.float32) + const)
    return out


def _get_nc(mode=MODE):
    if mode not in _CACHE:
        _CACHE[mode] = build_nc(mode)
    return _CACHE[mode]


def run(inputs, mode=MODE, trace=False, tmpdir=None):
    nc = _get_nc(mode)
    in_maps = host_prep(inputs, mode)
    res = bass_utils.run_bass_kernel_spmd(
        nc, in_maps, core_ids=list(range(8)), trace=trace, tmpdir=tmpdir)
    return host_finish(res.results, inputs), res


def kernel(**inputs):
    out, _ = run(inputs)
    return out


# revision 33
# speedup vs baseline: 1.0464x; 1.0464x over previous
"""Dinov3 ViT attention (B=4, N=1024, D=1024, H=16, HD=64) on 8 TRN2
NeuronCores, written against the Bass/Tile stack.

Sharding: core c -> (batch b = c//2, head-group g = c%2, 8 heads each).
Each core computes q/k/v projections for its 512-feature slice, rotary,
attention, and a partial o_proj (its head-group's wo columns). The host
sums the two partials per batch and adds the constant bias vector
(bo + bv @ wo.T - exact, since softmax rows sum to 1).

v3 schedule.  The TileScheduler greedily drains any ready work, so the
pipeline is enforced structurally: all score/projection/V/o_proj PSUM
tiles rotate through ONE 3-slot pool (6 banks) + a separate AV
accumulator slot (2 banks).  A tile grant waits for the release three
allocations back, which ties the PE's progress to the ACT exp stream
(each scores tile is released by its exp) - emission order therefore
becomes execution order with ~3 groups of slack, with no scheduler
hints needed.

  head     chunked DMAs; pair-0 Q/K projections emitted k-major so PE
           tracks the xt chunk arrivals; K copy on ACT, Q bias on DVE.
  phase m  per kv-block: scores S^T -> exp (ACT); AV(m-1) head-A lagged
           per-kb into the pav slot; Q/K(m+1) projection half-token
           chunks as filler; AV head-B as an end-of-phase block.
  phase 3  o_proj stage-1 (pairs 0,1 -> fp16 obuf via DVE) as filler.
  tail     AV(3) both heads (pav + big slot), finalize = reciprocal +
           cross-base mult (2 DVE ops/head), o_proj stage-2 per token
           block: identity-matmul folds obuf back into PSUM (PE), adds
           pairs 2,3, ACT copies to fp16, DMA out.

Host passes pre-transposed/sliced fp16 inputs; output is fp16, host
upcasts and reduces the two per-batch partials.
"""

import sys

if "/opt/trn_rl_repo" not in sys.path:
    sys.path.insert(0, "/opt/trn_rl_repo")

import numpy as np

import concourse.bass as bass
import concourse.bacc as bacc
import concourse.mybir as mybir
from concourse import tile
from concourse import bass_utils
from contextlib import ExitStack

B, N, D = 4, 1024, 1024
H, HD = 16, 64
F = 512          # per-core feature slice (8 heads)
P = 128
NKB = 8          # contraction blocks over D
NTB = 8          # token blocks of 128
NH = 8           # local heads
MODE = "f16"

_CACHE = {}


def build_nc(mode="f16", debug=False):
    assert mode in ("f16", "bf16")
    dt = mybir.dt.float16 if mode == "f16" else mybir.dt.bfloat16
    f32 = mybir.dt.float32
    AF = mybir.ActivationFunctionType
    ALU = mybir.AluOpType

    nc = bacc.Bacc("TRN2", target_bir_lowering=False, debug=False, num_devices=8)
    xt_d = nc.dram_tensor("xt", (P, NKB * N), dt, kind="ExternalInput").ap()
    wqp_d = nc.dram_tensor("wqp", (P, 4 * NKB * P), dt, kind="ExternalInput").ap()
    wkp_d = nc.dram_tensor("wkp", (P, 4 * NKB * P), dt, kind="ExternalInput").ap()
    wvt_d = nc.dram_tensor("wvt", (P, NKB * F), dt, kind="ExternalInput").ap()
    wot_d = nc.dram_tensor("wot", (P, 4 * D), dt, kind="ExternalInput").ap()
    bq_d = nc.dram_tensor("bq", (P, 4), f32, kind="ExternalInput").ap()
    cs_d = nc.dram_tensor("cs", (P, N), dt, kind="ExternalInput").ap()
    ss_d = nc.dram_tensor("ss", (P, N), dt, kind="ExternalInput").ap()
    id_d = nc.dram_tensor("ident", (P, P), dt, kind="ExternalInput").ap()
    out_d = nc.dram_tensor("out", (N, D), dt, kind="ExternalOutput").ap()

    with tile.TileContext(nc) as tc, ExitStack() as top:
        pool = top.enter_context(tc.tile_pool(name="sb", bufs=1))

        cs_sb = pool.tile([P, N], dt, name="cs")
        ss_sb = pool.tile([P, N], dt, name="ss")
        bq_sb = pool.tile([P, 4], f32, name="bq")
        ebias = pool.tile([P, 1], f32, name="ebias")
        id_sb = pool.tile([P, P], dt, name="ident")
        xt_big = pool.tile([P, NKB * N], dt, name="xtb")
        wq_big = pool.tile([P, 4 * NKB * P], dt, name="wqb")
        wk_big = pool.tile([P, 4 * NKB * P], dt, name="wkb")
        wv_big = pool.tile([P, NKB * F], dt, name="wvb")
        wot_big = pool.tile([P, 4 * D], dt, name="wotb")
        xt_sb = [xt_big[:, k * N:(k + 1) * N] for k in range(NKB)]
        wv_sb = [wv_big[:, k * F:(k + 1) * F] for k in range(NKB)]
        wot_sb = [wot_big[:, m * D:(m + 1) * D] for m in range(4)]
        qt_sb = [pool.tile([P, N], dt, name=f"qt{m}") for m in range(4)]
        kt_sb = [pool.tile([P, N], dt, name=f"kt{m}") for m in range(4)]
        v65_sb = [pool.tile([P, NH * 128], dt, name=f"v65_{t}") for t in range(NTB)]
        ot_sb = [pool.tile([P, N], dt, name=f"ot{m}") for m in range(4)]
        obuf = [pool.tile([P, N], dt, name=f"ob{qb}") for qb in range(NTB)]

        nc.any.memset(ebias[:], -3.0)
        etbl = pool.tile([1, 1], f32, name="etbl")
        nc.scalar.activation(etbl[:], ebias[0:1, 0:1], AF.Exp)  # preload table

        # ---- DMA kickoff: xt across all 3 queues (k-major head MMs
        # track the chunk sems); gpsimd frees early for rotary swaps.
        WPC = NKB * P  # columns per pair in wq/wk packs
        nc.sync.dma_start(bq_sb[:], bq_d)
        nc.sync.dma_start(id_sb[:], id_d)
        nc.sync.dma_start(wq_big[:, 0:WPC], wqp_d[:, 0:WPC])
        nc.sync.dma_start(wk_big[:, 0:WPC], wkp_d[:, 0:WPC])
        nc.scalar.dma_start(xt_big[:, 0:3 * N], xt_d[:, 0:3 * N])
        nc.gpsimd.dma_start(xt_big[:, 3 * N:6 * N], xt_d[:, 3 * N:6 * N])
        nc.sync.dma_start(cs_sb[:], cs_d)
        nc.sync.dma_start(ss_sb[:], ss_d)
        nc.sync.dma_start(xt_big[:, 6 * N:8 * N], xt_d[:, 6 * N:8 * N])
        nc.sync.dma_start(wv_big[:], wvt_d)
        for m in range(1, 4):
            nc.scalar.dma_start(wq_big[:, m * WPC:(m + 1) * WPC],
                                wqp_d[:, m * WPC:(m + 1) * WPC])
            nc.scalar.dma_start(wk_big[:, m * WPC:(m + 1) * WPC],
                                wkp_d[:, m * WPC:(m + 1) * WPC])
        nc.scalar.dma_start(wot_big[:], wot_d)

        swp = top.enter_context(tc.tile_pool(name="swp", bufs=2))
        ptp = top.enter_context(tc.tile_pool(name="ptp", bufs=32))
        rcpp = top.enter_context(tc.tile_pool(name="rcpp", bufs=2))
        ost = top.enter_context(tc.tile_pool(name="ost", bufs=3))

        def wsl(w_big, m, k):
            return w_big[:, (m * NKB + k) * P:(m * NKB + k + 1) * P]

        def proj_half(m, which, half, big):
            """Half-token-range Q/K projection chunk for pair m (8 MMs)."""
            w_big = wq_big if which == "q" else wk_big
            dst = qt_sb[m] if which == "q" else kt_sb[m]
            hs = slice(half * F, (half + 1) * F)
            ps = big.tile([P, F], f32, tag="big", name="psp")
            for k in range(NKB):
                nc.tensor.matmul(ps[:], wsl(w_big, m, k), xt_sb[k][:, hs],
                                 start=(k == 0), stop=(k == NKB - 1))
            if which == "q":
                nc.vector.tensor_scalar_add(dst[:, hs], ps[:],
                                            bq_sb[:, m:m + 1])
            else:
                nc.vector.tensor_copy(dst[:, hs], ps[:])

        def rotary(m, which, crit=False):
            """In-place rotary on qt/kt[m]: src = src*cs + swap(src)*ss.

            crit: head critical path - swaps on 2 queues, each element-wise
            op column-split across DVE and gpsimd to halve chain latency.
            """
            src_sb = qt_sb if which == "q" else kt_sb
            sw = swp.tile([P, N], dt, tag="sw", name="sw")
            for blk in range(4):
                o = blk * 32
                eng = nc.sync if (crit and blk % 2) else nc.gpsimd
                eng.dma_start(sw[o:o + 32, :],
                              src_sb[m][o ^ 32:(o ^ 32) + 32, :])
            if crit:
                hv, hg = slice(0, F), slice(F, N)
                for dst, in0, in1 in (
                        (sw, sw, ss_sb),
                        (src_sb[m], src_sb[m], cs_sb)):
                    nc.vector.tensor_tensor(dst[:, hv], in0[:, hv],
                                            in1[:, hv], op=ALU.mult)
                    nc.gpsimd.tensor_tensor(dst[:, hg], in0[:, hg],
                                            in1[:, hg], op=ALU.mult)
                nc.vector.tensor_tensor(src_sb[m][:, hv], src_sb[m][:, hv],
                                        sw[:, hv], op=ALU.add)
                nc.gpsimd.tensor_tensor(src_sb[m][:, hg], src_sb[m][:, hg],
                                        sw[:, hg], op=ALU.add)
            else:
                nc.gpsimd.tensor_tensor(sw[:], sw[:], ss_sb[:], op=ALU.mult)
                nc.vector.tensor_tensor(src_sb[m][:], src_sb[m][:], cs_sb[:],
                                        op=ALU.mult)
                nc.vector.tensor_tensor(src_sb[m][:], src_sb[m][:], sw[:],
                                        op=ALU.add)

        def scores_kb(m, kb, big, pts):
            """S^T for pair m, kv-block kb -> exp -> pts tiles (fp16)."""
            ps2 = [big.tile([P, N], f32, tag="big", name="pss")
                   for _ in range(2)]
            for qh in range(2):
                qs = slice(qh * F, (qh + 1) * F)
                for par in range(2):
                    off = par * 64
                    nc.tensor.matmul(
                        ps2[par][:, qs],
                        kt_sb[m][off:off + 64, kb * P:(kb + 1) * P],
                        qt_sb[m][off:off + 64, qs],
                        start=True, stop=True, tile_position=(off, 0))
            for par in range(2):
                ptile = ptp.tile([P, N], dt, tag="pt", name="ptile", bufs=32)
                nc.scalar.activation(ptile[:], ps2[par][:], AF.Exp,
                                     scale=0.125, bias=ebias[:])
                pts[par][kb] = ptile

        def av_kb(m, par, kb, av, pts):
            """AV accumulation MMs for head 2m+par, kv-block kb."""
            h = 2 * m + par
            for qh in range(2):
                qs = slice(qh * F, (qh + 1) * F)
                nc.tensor.matmul(av[:, qs],
                                 v65_sb[kb][:, h * 128:(h + 1) * 128],
                                 pts[par][kb][:, qs],
                                 start=(kb == 0), stop=(kb == NTB - 1))

        def av_finalize(m, par, av):
            """ot rows for head 2m+par = av[64:128] * recip(av[0:64])."""
            off = par * 64
            rcp = rcpp.tile([64, N], f32, tag="rcp", name="rcp")
            nc.vector.reciprocal_approx_fast(rcp[:], av[0:64, :])
            nc.vector.tensor_tensor(ot_sb[m][off:off + 64, :],
                                    av[64:128, :], rcp[:], op=ALU.mult)

        def v_proj(t, big):
            """V projection for token block t -> v65 (128-wide head slots)."""
            ps = big.tile([P, F], f32, tag="big", name="psv")
            for k in range(NKB):
                nc.tensor.matmul(ps[:], xt_sb[k][:, t * P:(t + 1) * P],
                                 wv_sb[k][:], start=(k == 0),
                                 stop=(k == NKB - 1))
            nc.any.memset(v65_sb[t][:], 1.0)
            dst = v65_sb[t][:].rearrange("p (h e) -> p h e", e=128)[:, :, 64:128]
            nc.vector.tensor_copy(dst, ps[:].rearrange("p (h e) -> p h e", e=64))

        def oproj_stage1(qb, big):
            """o_proj partial (pairs 0,1) for token block qb -> fp16 obuf."""
            ps = big.tile([P, D], f32, tag="big", name="ps1")
            for i, mm_ in enumerate((0, 1)):
                for half in range(2):
                    hs = slice(half * F, (half + 1) * F)
                    nc.tensor.matmul(ps[:, hs],
                                     ot_sb[mm_][:, qb * P:(qb + 1) * P],
                                     wot_sb[mm_][:, hs],
                                     start=(i == 0), stop=(i == 1))
            if qb >= 5:
                nc.scalar.copy(obuf[qb][:], ps[:])
            else:
                nc.vector.tensor_copy(obuf[qb][:], ps[:])

        def oproj_s2a(qb, big):
            """Stage-2 preload: obuf (identity-MM) + pair 2 into PSUM."""
            ps = big.tile([P, D], f32, tag="big", name="ps2")
            for half in range(2):
                hs = slice(half * F, (half + 1) * F)
                nc.tensor.matmul(ps[:, hs], id_sb[:], obuf[qb][:, hs],
                                 start=True, stop=False)
            for half in range(2):
                hs = slice(half * F, (half + 1) * F)
                nc.tensor.matmul(ps[:, hs],
                                 ot_sb[2][:, qb * P:(qb + 1) * P],
                                 wot_sb[2][:, hs], start=False, stop=False)
            return ps

        def oproj_s2b(qb, ps):
            """Stage-2 finish: pair 3, ACT copy to fp16, DMA out."""
            for half in range(2):
                hs = slice(half * F, (half + 1) * F)
                nc.tensor.matmul(ps[:, hs],
                                 ot_sb[3][:, qb * P:(qb + 1) * P],
                                 wot_sb[3][:, hs], start=False, stop=True)
            oout = ost.tile([P, D], dt, tag="oout", name="oout")
            if qb % 2 == 0:
                nc.scalar.copy(oout[:], ps[:])
            else:
                nc.vector.tensor_copy(oout[:], ps[:])
            eng = nc.sync if qb % 2 == 0 else nc.scalar
            eng.dma_start(out_d[qb * P:(qb + 1) * P, :], oout[:])

        pts = [[[None] * NTB, [None] * NTB] for _ in range(4)]

        with ExitStack() as st_att:
            big = st_att.enter_context(tc.tile_pool(name="big", bufs=3,
                                                    space="PSUM"))
            pav = st_att.enter_context(tc.tile_pool(name="pav", bufs=1,
                                                    space="PSUM"))

            # ---------- head: pair-0 Q/K projections, k-major ----------
            psQ = big.tile([P, N], f32, tag="big", name="psQ0")
            psK = big.tile([P, N], f32, tag="big", name="psK0")
            for k in range(NKB):
                for half in range(2):
                    hs = slice(half * F, (half + 1) * F)
                    nc.tensor.matmul(psQ[:, hs], wsl(wq_big, 0, k),
                                     xt_sb[k][:, hs],
                                     start=(k == 0), stop=(k == NKB - 1))
                    nc.tensor.matmul(psK[:, hs], wsl(wk_big, 0, k),
                                     xt_sb[k][:, hs],
                                     start=(k == 0), stop=(k == NKB - 1))
            nc.vector.tensor_scalar_add(qt_sb[0][:], psQ[:], bq_sb[:, 0:1])
            nc.scalar.copy(kt_sb[0][:], psK[:])
            rotary(0, "q", crit=True)
            rotary(0, "k", crit=True)

            # ---------- phase 0: scores(0) + V proj t0-5 + QK(1) ----------
            # V t0/t1 ahead of the first scores: fills PE during the
            # pair-0 rotary chain instead of head-of-line blocking on it.
            v_proj(0, big)
            v_proj(1, big)
            for kb in range(NTB):
                scores_kb(0, kb, big, pts[0])
                if 2 <= kb < 6:
                    v_proj(kb, big)
                if kb == 0:
                    proj_half(1, "q", 0, big)
                elif kb == 1:
                    proj_half(1, "q", 1, big)
                elif kb == 2:
                    rotary(1, "q")
                    proj_half(1, "k", 0, big)
                elif kb == 3:
                    proj_half(1, "k", 1, big)
                elif kb == 4:
                    rotary(1, "k")

            # ---------- phases 1..2 ----------
            for m in (1, 2):
                avA = pav.tile([P, N], f32, tag="pav", name="avA")
                for kb in range(NTB):
                    scores_kb(m, kb, big, pts[m])
                    av_kb(m - 1, 0, kb, avA, pts[m - 1])
                    if m == 1 and kb in (0, 2):
                        v_proj(6 + kb // 2, big)
                    if kb == 0:
                        proj_half(m + 1, "q", 0, big)
                    elif kb == 1:
                        proj_half(m + 1, "q", 1, big)
                    elif kb == 2:
                        rotary(m + 1, "q")
                        proj_half(m + 1, "k", 0, big)
                    elif kb == 3:
                        proj_half(m + 1, "k", 1, big)
                    elif kb == 4:
                        rotary(m + 1, "k")
                av_finalize(m - 1, 0, avA)
                avB = pav.tile([P, N], f32, tag="pav", name="avB")
                for kb in range(NTB):
                    av_kb(m - 1, 1, kb, avB, pts[m - 1])
                av_finalize(m - 1, 1, avB)

            # ---------- phase 3: o_proj stage-1 as filler ----------
            avA = pav.tile([P, N], f32, tag="pav", name="avA3")
            for kb in range(NTB):
                scores_kb(3, kb, big, pts[3])
                av_kb(2, 0, kb, avA, pts[2])
                oproj_stage1(kb, big)
            av_finalize(2, 0, avA)
            avB = pav.tile([P, N], f32, tag="pav", name="avB3")
            for kb in range(NTB):
                av_kb(2, 1, kb, avB, pts[2])
            av_finalize(2, 1, avB)

            # ---------- tail: AV(3) + o_proj stage-2 ----------
            # AV(3) accumulators both in the big pool: their MMs are
            # pts-gated only, so they run concurrently with the AVB(2)
            # block (pav) instead of serializing behind its finalize.
            avA3 = big.tile([P, N], f32, tag="big", name="avA4")
            avB3 = big.tile([P, N], f32, tag="big", name="avB4")
            av_kb(3, 0, 0, avA3, pts[3])
            av_kb(3, 1, 0, avB3, pts[3])
            avB2 = pav.tile([P, N], f32, tag="pav", name="avB2b")
            for k2 in range(NTB):
                av_kb(2, 1, k2, avB2, pts[2])
            av_finalize(2, 1, avB2)
            for kb in range(1, NTB - 1):
                av_kb(3, 0, kb, avA3, pts[3])
                av_kb(3, 1, kb, avB3, pts[3])
            av_kb(3, 0, NTB - 1, avA3, pts[3])
            av_finalize(3, 0, avA3)
            av_kb(3, 1, NTB - 1, avB3, pts[3])
            av_finalize(3, 1, avB3)
            ps2 = {0: oproj_s2a(0, big)}
            for qb in range(NTB):
                oproj_s2b(qb, ps2.pop(qb))
                if qb + 1 < NTB:
                    ps2[qb + 1] = oproj_s2a(qb + 1, big)

    nc.compile()
    return nc


def host_prep(inputs, mode=MODE):
    """Slice/transpose full inputs into 8 per-core input maps."""
    hs = np.asarray(inputs["hidden_states"], np.float32)
    cos = np.asarray(inputs["cos"], np.float32)
    sin = np.asarray(inputs["sin"], np.float32)
    wq = np.asarray(inputs["wq"], np.float32)
    wk = np.asarray(inputs["wk"], np.float32)
    wv = np.asarray(inputs["wv"], np.float32)
    wo = np.asarray(inputs["wo"], np.float32)
    bq = np.asarray(inputs["bq"], np.float32)

    if mode == "bf16":
        import ml_dtypes
        cast = lambda a: np.ascontiguousarray(a).astype(ml_dtypes.bfloat16)
    else:
        cast = lambda a: np.ascontiguousarray(a).astype(np.float16)

    sgn = np.ones((64, 1), np.float32)
    sgn[:32] = -1.0
    ident = np.eye(P, dtype=np.float32)
    in_maps = []
    for c in range(8):
        b, g = c // 2, c % 2
        fs = slice(g * F, (g + 1) * F)
        csx = cos[b].T  # (64, N)
        ssx = sin[b].T * sgn
        pack = lambda a, kd: np.ascontiguousarray(
            a.reshape(kd, P, -1).transpose(1, 0, 2).reshape(P, -1))

        def pack_pairs(w):
            # w: (512 out-features, 1024 D) -> [P, 4*NKB*P] pair-major
            wt = w.T  # (1024 D, 512 feat)
            blocks = []
            for m in range(4):
                blocks.append(pack(wt[:, m * P:(m + 1) * P], NKB))
            return np.concatenate(blocks, axis=1)

        in_maps.append({
            "xt": cast(pack(hs[b].T, NKB)),
            "wqp": cast(pack_pairs(wq[fs, :])),
            "wkp": cast(pack_pairs(wk[fs, :])),
            "wvt": cast(pack(wv[fs, :].T, NKB)),
            "wot": cast(pack(wo[:, fs].T, 4)),
            "bq": np.ascontiguousarray(bq[fs].reshape(4, P).T, np.float32),
            "cs": cast(np.concatenate([csx, csx], axis=0)),
            "ss": cast(np.concatenate([ssx, ssx], axis=0)),
            "ident": cast(ident),
        })
    return in_maps


def host_finish(results, inputs):
    bo = np.asarray(inputs["bo"], np.float32)
    bv = np.asarray(inputs["bv"], np.float32)
    wo = np.asarray(inputs["wo"], np.float32)
    const = bo + bv @ wo.T
    out = np.empty((B, N, D), np.float32)
    for b in range(B):
        out[b] = (results[2 * b]["out"].astype(np.float32)
                  + results[2 * b + 1]["out"].astype(np.float32) + const)
    return out


def _get_nc(mode=MODE):
    if mode not in _CACHE:
        _CACHE[mode] = build_nc(mode)
    return _CACHE[mode]


def run(inputs, mode=MODE, trace=False, tmpdir=None):
    nc = _get_nc(mode)
    in_maps = host_prep(inputs, mode)
    res = bass_utils.run_bass_kernel_spmd(
        nc, in_maps, core_ids=list(range(8)), trace=trace, tmpdir=tmpdir)
    return host_finish(res.results, inputs), res


def kernel(**inputs):
    out, _ = run(inputs)
    return out
